# revision 1
# baseline (speedup 1.0000x reference)
"""Trainium2 Bass kernel for nn_Attention (GroupNorm + single-head-dim attention + proj).

Reference computation (B=16, C=256, H=W=32, nh=4, d=64, groups=8):
    h = group_norm(x, norm_w, norm_b)
    qkv = qkv_w @ h + qkv_b          (1x1 conv == channel matmul)
    q, k, v = split(qkv)             [B, nh, d, N], N = H*W = 1024
    attn = softmax(q^T k / sqrt(d))  over keys m
    out = v @ attn^T                 [B, nh, d, N]
    y = x + proj_w @ out + proj_b

Sharding: data-parallel over batch, 2 batches per core x 8 cores (SPMD, one NEFF).

Per-core layout choices:
  - All big matmuls in float32r (full PE rate, ~1.6e-4 rel err).
  - Attention computed in S^T = k^T q layout [m, n] so the AV contraction
    (over m) needs no transposes anywhere.
  - Softmax rowsums come free from a shared ones-block in the AV stationary
    operand ([v_lo | ones | v_hi] -> lhsT [v_lo|ones] puts head-lo out on
    partitions 0:64 and its rowsum replicated on 64:128; lhsT [ones|v_hi]
    mirrors that for head-hi).
  - Normalization: reciprocal_approx_fast on the replicated rowsum lanes,
    DMA partition-shift to the output lanes, one tensor-tensor multiply.
"""
import numpy as np

B, C, HW = 16, 256, 1024
NH, D, NG = 4, 64, 8
EPS = 1e-5
NCORES = 8
BPC = B // NCORES  # batches per core

_CACHE = {}


def _build_module(reps=1):
    import concourse.bacc as bacc
    import concourse.mybir as mybir
    from concourse import tile

    f32 = mybir.dt.float32
    f32r = mybir.dt.float32r
    AF = mybir.ActivationFunctionType

    nc = bacc.Bacc("TRN2", target_bir_lowering=False, num_devices=NCORES)

    x_d = nc.dram_tensor("x", [BPC, C, HW], f32, kind="ExternalInput")
    qkvwT_d = nc.dram_tensor("qkvwT", [C, 3 * C], f32, kind="ExternalInput")
    projwT_d = nc.dram_tensor("projwT", [C, C], f32, kind="ExternalInput")
    qkb_d = nc.dram_tensor("qkb", [2 * C], f32, kind="ExternalInput")
    pb2_d = nc.dram_tensor("pb2", [C], f32, kind="ExternalInput")
    nw_d = nc.dram_tensor("nw", [C], f32, kind="ExternalInput")
    nb_d = nc.dram_tensor("nb", [C], f32, kind="ExternalInput")
    y_d = nc.dram_tensor("y", [BPC, C, HW], f32, kind="ExternalOutput")

    # constants: group indicator matrices + ones block
    # chunk ch covers channels [128*ch, 128*ch+128) -> groups [4*ch, 4*ch+4)
    g_np = np.zeros((2, 128, NG), np.float32)
    gb_np = np.zeros((2, NG, 128), np.float32)
    for ch in range(2):
        for c in range(128):
            g = 4 * ch + c // 32
            g_np[ch, c, g] = 1.0
            gb_np[ch, g, c] = 1.0
    g_dram = nc.inline_tensor(np.ascontiguousarray(g_np), name="g_const")
    gb_dram = nc.inline_tensor(np.ascontiguousarray(gb_np), name="gb_const")
    ones_dram = nc.inline_tensor(np.ones((128, 64), np.float32), name="ones_const")

    with tile.TileContext(nc) as tc:
        with (
            tc.tile_pool(name="wp", bufs=1) as wp,        # weights/consts, persistent
            tc.tile_pool(name="big", bufs=1) as big,      # per-batch persistent tiles
            tc.tile_pool(name="tmp", bufs=3) as tmp,      # small transient tiles
            tc.tile_pool(name="es_p", bufs=4) as es_p,    # exp(S^T) tiles
            tc.tile_pool(name="rec_p", bufs=3) as rec_p,  # recip tiles
            tc.tile_pool(name="y_p", bufs=2) as y_p,      # output staging
            tc.tile_pool(name="x_p", bufs=2) as x_p,      # input, double-buffered across reps
            tc.tile_pool(name="ps_s", bufs=2, space="PSUM") as ps_s,    # 4 banks
            tc.tile_pool(name="ps_av", bufs=2, space="PSUM") as ps_av,  # 4 banks
        ):
            # ---------------- weights / constants ----------------
            qkvwT32 = wp.tile([128, 2, 3 * C], f32)
            projwT32 = wp.tile([128, 2, C], f32)
            for ch in range(2):
                nc.gpsimd.dma_start(qkvwT32[:, ch, :], qkvwT_d[128 * ch:128 * (ch + 1), :])
                nc.gpsimd.dma_start(projwT32[:, ch, :], projwT_d[128 * ch:128 * (ch + 1), :])
            qkvwT = wp.tile([128, 2, 3 * C], f32r)
            projwT = wp.tile([128, 2, C], f32r)
            nc.vector.tensor_copy(qkvwT[:], qkvwT32[:])
            nc.vector.tensor_copy(projwT[:], projwT32[:])

            qkb = wp.tile([128, 4], f32)
            nc.gpsimd.dma_start(qkb[:], qkb_d.rearrange("(t p) -> p t", p=128))
            pb2 = wp.tile([128, 2], f32)
            nc.gpsimd.dma_start(pb2[:], pb2_d.rearrange("(t p) -> p t", p=128))
            nw = wp.tile([128, 2], f32)
            nc.gpsimd.dma_start(nw[:], nw_d.rearrange("(t p) -> p t", p=128))
            nb = wp.tile([128, 2], f32)
            nc.gpsimd.dma_start(nb[:], nb_d.rearrange("(t p) -> p t", p=128))

            g_c = wp.tile([128, 2, NG], f32)
            nc.gpsimd.dma_start(g_c[:], g_dram[:].rearrange("c p g -> p c g"))
            gb_c = wp.tile([NG, 2, 128], f32)
            nc.gpsimd.dma_start(gb_c[:], gb_dram[:].rearrange("c p g -> p c g"))
            ones32 = wp.tile([128, 64], f32)
            nc.gpsimd.dma_start(ones32[:], ones_dram[:])
            ones_r = wp.tile([128, 64], f32r)
            nc.vector.tensor_copy(ones_r[:], ones32[:])
            eps_t = wp.tile([128, 1], f32)
            nc.vector.memset(eps_t[:], EPS)

            # persistent per-batch tiles
            h_t = big.tile([128, BPC, 2, HW], f32r)      # groupnorm output
            qk_t = big.tile([128, BPC, 4, HW], f32r)     # q01,q23,k01,k23
            vtp_t = big.tile([128, BPC, 2, 8, 192], f32r)  # [v_lo|ones|v_hi] per (hp, m-tile)
            on_t = big.tile([128, BPC, 2, HW], f32r)     # normalized attn out (pre-proj)

            for rep in range(reps):
                x_t = x_p.tile([128, BPC, 2, HW], f32, name=f"x_{rep}", tag="x")
                # ---------------- phase 1: GN + QKV per batch ----------------
                for b in range(BPC):
                    for ch in range(2):
                        nc.sync.dma_start(x_t[:, b, ch, :], x_d[b, 128 * ch:128 * (ch + 1), :])

                    # per-channel stats -> per-group via PE -> broadcast back
                    g_ps = ps_av.tile([NG, 2], f32, name=f"g_ps_{b}", tag="av")
                    st2s = []
                    for ch in range(2):
                        st6 = tmp.tile([128, 2, 6], f32, name=f"st6_{b}_{ch}", tag="st6")
                        for i in range(2):
                            nc.vector.bn_stats(st6[:, i, :], x_t[:, b, ch, 512 * i:512 * (i + 1)])
                        mv = tmp.tile([128, 2], f32, name=f"mv_{b}_{ch}", tag="mv")
                        nc.vector.bn_aggr(mv[:], st6[:])
                        st2 = tmp.tile([128, 2], f32, name=f"st2_{b}_{ch}", tag="st2")
                        nc.gpsimd.tensor_copy(st2[:, 0:1], mv[:, 0:1])
                        sq = tmp.tile([128, 1], f32, name=f"sq_{b}_{ch}", tag="sq")
                        nc.vector.tensor_mul(sq[:], mv[:, 0:1], mv[:, 0:1])
                        nc.vector.tensor_add(st2[:, 1:2], mv[:, 1:2], sq[:])
                        st2s.append(st2)
                    for ch in range(2):
                        nc.tensor.matmul(g_ps[:], g_c[:, ch, :], st2s[ch][:],
                                         start=(ch == 0), stop=(ch == 1))
                    gst = tmp.tile([NG, 2], f32, name=f"gst_{b}", tag="gst")
                    nc.vector.tensor_copy(gst[:], g_ps[:])

                    for ch in range(2):
                        bc_ps = ps_av.tile([128, 2], f32, name=f"bc_ps_{b}_{ch}", tag="av")
                        nc.tensor.matmul(bc_ps[:], gb_c[:, ch, :], gst[:],
                                         start=True, stop=True)
                        mean_c = tmp.tile([128, 1], f32, name=f"mean_{b}_{ch}", tag="mean")
                        ex2_c = tmp.tile([128, 1], f32, name=f"ex2_{b}_{ch}", tag="ex2")
                        nc.vector.tensor_scalar_mul(mean_c[:], bc_ps[:, 0:1], 1.0 / 32.0)
                        nc.vector.tensor_scalar_mul(ex2_c[:], bc_ps[:, 1:2], 1.0 / 32.0)
                        var_c = tmp.tile([128, 1], f32, name=f"var_{b}_{ch}", tag="var")
                        nc.vector.tensor_mul(var_c[:], mean_c[:], mean_c[:])
                        nc.vector.tensor_sub(var_c[:], ex2_c[:], var_c[:])
                        sd = tmp.tile([128, 1], f32, name=f"sd_{b}_{ch}", tag="sd")
                        nc.scalar.activation(sd[:], var_c[:], AF.Sqrt, bias=eps_t[:])
                        nc.vector.reciprocal(sd[:], sd[:])
                        a_c = tmp.tile([128, 1], f32, name=f"a_{b}_{ch}", tag="a_c")
                        nc.vector.tensor_mul(a_c[:], sd[:], nw[:, ch:ch + 1])
                        b_c = tmp.tile([128, 1], f32, name=f"b_{b}_{ch}", tag="b_c")
                        nc.vector.tensor_mul(b_c[:], mean_c[:], a_c[:])
                        nc.vector.tensor_sub(b_c[:], nb[:, ch:ch + 1], b_c[:])
                        nc.vector.tensor_scalar(
                            out=h_t[:, b, ch, :], in0=x_t[:, b, ch, :],
                            scalar1=a_c[:], scalar2=b_c[:],
                            op0=mybir.AluOpType.mult, op1=mybir.AluOpType.add)

                    # q01,q23,k01,k23 o-tiles: qkv rows [0,512)
                    for t in (0, 2):
                        qk_ps = ps_s.tile([128, HW], f32, name=f"qk_ps_{b}_{t}", tag="s")
                        for half in range(2):
                            for ch in range(2):
                                nc.tensor.matmul(
                                    qk_ps[:, 512 * half:512 * (half + 1)],
                                    qkvwT[:, ch, 128 * t:128 * (t + 1)],
                                    h_t[:, b, ch, 512 * half:512 * (half + 1)],
                                    start=(ch == 0), stop=(ch == 1))
                        nc.vector.tensor_scalar_add(qk_t[:, b, t, :], qk_ps[:], qkb[:, t:t + 1])

                    # vT' tiles: v^T = h^T @ Wv^T computed per m-tile (n-tile of N)
                    for nt in range(8):
                        vt_ps = ps_av.tile([128, 256], f32, name=f"vt_ps_{b}_{nt}", tag="av")
                        for ch in range(2):
                            nc.tensor.matmul(
                                vt_ps[:],
                                h_t[:, b, ch, 128 * nt:128 * (nt + 1)],
                                qkvwT[:, ch, 2 * C:3 * C],
                                start=(ch == 0), stop=(ch == 1))
                        for hp in range(2):
                            # [v_lo | ones | v_hi]; ones written once below
                            nc.vector.tensor_copy(
                                vtp_t[:, b, hp, nt, :].rearrange("p (s c) -> p s c", s=3)[:, 0::2, :],
                                vt_ps[:, 128 * hp:128 * (hp + 1)].rearrange("p (s c) -> p s c", s=2))
                            nc.gpsimd.tensor_copy(vtp_t[:, b, hp, nt, 64:128], ones_r[:])


                    # q23/k23 after vtp so attention(hp01) can start earlier
                    for t in (1, 3):
                        qk_ps = ps_s.tile([128, HW], f32, name=f"qk_ps2_{b}_{t}", tag="s")
                        for half in range(2):
                            for ch in range(2):
                                nc.tensor.matmul(
                                    qk_ps[:, 512 * half:512 * (half + 1)],
                                    qkvwT[:, ch, 128 * t:128 * (t + 1)],
                                    h_t[:, b, ch, 512 * half:512 * (half + 1)],
                                    start=(ch == 0), stop=(ch == 1))
                        nc.vector.tensor_scalar_add(qk_t[:, b, t, :], qk_ps[:], qkb[:, t:t + 1])

                # ---------------- phase 2: attention per (b, head-pair) ----------------
                for b in range(BPC):
                    for hp in range(2):
                        q_ap = qk_t[:, b, hp, :]
                        k_ap = qk_t[:, b, 2 + hp, :]
                        for half in range(2):
                            av_ps = ps_av.tile([128, HW], f32, name=f"av_{b}_{hp}_{half}", tag="av")
                            for m in range(8):
                                s_ps = ps_s.tile([128, HW], f32, name=f"s_{b}_{hp}_{half}_{m}", tag="s")
                                nc.tensor.matmul(
                                    s_ps[:, 0:512],
                                    k_ap[0:64, 128 * m:128 * (m + 1)],
                                    q_ap[0:64, 512 * half:512 * (half + 1)],
                                    start=True, stop=True)
                                nc.tensor.matmul(
                                    s_ps[:, 512:1024],
                                    k_ap[64:128, 128 * m:128 * (m + 1)],
                                    q_ap[64:128, 512 * half:512 * (half + 1)],
                                    start=True, stop=True)
                                es = es_p.tile([128, HW], f32r, name=f"es_{b}_{hp}_{half}_{m}",
                                               tag="es")
                                nc.scalar.activation(es[:], s_ps[:], AF.Exp, scale=0.125)
                                nc.tensor.matmul(
                                    av_ps[:, 0:512], vtp_t[:, b, hp, m, 0:128], es[:, 0:512],
                                    start=(m == 0), stop=(m == 7))
                                nc.tensor.matmul(
                                    av_ps[:, 512:1024], vtp_t[:, b, hp, m, 64:192], es[:, 512:1024],
                                    start=(m == 0), stop=(m == 7))
                            # normalize: R_lo on rows 64:128 of cols 0:512,
                            #            R_hi on rows 0:64 of cols 512:1024
                            rec = rec_p.tile([128, HW], f32, name=f"rl_{b}_{hp}_{half}",
                                             tag="rl")
                            nc.vector.reciprocal_approx_fast(rec[:], av_ps[:])
                            recs = rec_p.tile([128, 512], f32, name=f"rs_{b}_{hp}_{half}",
                                              tag="rs")
                            nc.sync.dma_start(recs[0:64, :], rec[64:128, 0:512])
                            nc.sync.dma_start(recs[64:128, :], rec[0:64, 512:1024])
                            nc.vector.tensor_mul(
                                on_t[0:64, b, hp, 512 * half:512 * (half + 1)],
                                av_ps[0:64, 0:512], recs[0:64, :])
                            nc.vector.tensor_mul(
                                on_t[64:128, b, hp, 512 * half:512 * (half + 1)],
                                av_ps[64:128, 512:1024], recs[64:128, :])

                # ---------------- phase 3: proj + residual ----------------
                for b in range(BPC):
                    for ot in range(2):
                        y_ps = ps_s.tile([128, HW], f32, name=f"y_ps_{b}_{ot}", tag="s")
                        for half in range(2):
                            for ch in range(2):
                                nc.tensor.matmul(
                                    y_ps[:, 512 * half:512 * (half + 1)],
                                    projwT[:, ch, 128 * ot:128 * (ot + 1)],
                                    on_t[:, b, ch, 512 * half:512 * (half + 1)],
                                    start=(ch == 0), stop=(ch == 1))
                        y_sb = y_p.tile([128, HW], f32, name=f"y_sb_{b}_{ot}", tag="y")
                        nc.vector.scalar_tensor_tensor(
                            out=y_sb[:], in0=y_ps[:], scalar=pb2[:, ot:ot + 1],
                            in1=x_t[:, b, ot, :],
                            op0=mybir.AluOpType.add, op1=mybir.AluOpType.add)
                        nc.sync.dma_start(y_d[b, 128 * ot:128 * (ot + 1), :], y_sb[:])

    nc.finalize()
    return nc


def _prep_inputs(x, norm_w, norm_b, qkv_w, qkv_b, proj_w, proj_b):
    x = np.asarray(x, np.float32).reshape(B, C, HW)
    qkv_w = np.asarray(qkv_w, np.float32)
    qkv_b = np.asarray(qkv_b, np.float32)
    proj_w = np.asarray(proj_w, np.float32)
    proj_b = np.asarray(proj_b, np.float32)
    qkvwT = np.ascontiguousarray(qkv_w.T)
    projwT = np.ascontiguousarray(proj_w.T)
    qkb = np.ascontiguousarray(qkv_b[:2 * C])
    # v-bias and proj bias folded: y += proj_w @ (out + v_bias) + proj_b
    pb2 = (np.asarray(proj_b, np.float64)
           + np.asarray(proj_w, np.float64) @ np.asarray(qkv_b[2 * C:], np.float64)
           ).astype(np.float32)
    shared = {
        "qkvwT": qkvwT, "projwT": projwT, "qkb": qkb, "pb2": pb2,
        "nw": np.ascontiguousarray(np.asarray(norm_w, np.float32)),
        "nb": np.ascontiguousarray(np.asarray(norm_b, np.float32)),
    }
    in_maps = []
    for i in range(NCORES):
        m = {"x": np.ascontiguousarray(x[BPC * i:BPC * (i + 1)])}
        m.update(shared)
        in_maps.append(m)
    return in_maps


def kernel(x, norm_w, norm_b, qkv_w, qkv_b, proj_w, proj_b, _profile=False, _reps=1):
    from concourse.bass_utils import run_bass_kernel_spmd

    key = ("nc", _reps)
    if key not in _CACHE:
        _CACHE[key] = _build_module(reps=_reps)
    nc = _CACHE[key]

    in_maps = _prep_inputs(x, norm_w, norm_b, qkv_w, qkv_b, proj_w, proj_b)
    res = run_bass_kernel_spmd(nc, in_maps, core_ids=list(range(NCORES)),
                               trace=_profile)
    y = np.concatenate([r["y"] for r in res.results], axis=0)
    y = y.reshape(B, C, 32, 32)
    if _profile:
        return y, res
    return y



# revision 15
# speedup vs baseline: 3.0313x; 3.0313x over previous
"""Trainium2 Bass kernel for nn_Attention (GroupNorm + attention + proj).

Reference computation (B=16, C=256, H=W=32, nh=4, d=64, groups=8):
    h = group_norm(x, norm_w, norm_b)
    qkv = qkv_w @ h + qkv_b          (1x1 conv == channel matmul)
    q, k, v = split(qkv)             [B, nh, d, N], N = H*W = 1024
    attn = softmax(q^T k / sqrt(d))  over keys m
    out = v @ attn^T                 [B, nh, d, N]
    y = x + proj_w @ out + proj_b
Sharding: data-parallel over batch, 2 batches per core x 8 cores (SPMD, one NEFF).

Per-core layout (same math as v1 baseline):
  - All big matmuls in float32r (full PE rate at >=256 moving cols).
  - Attention computed in S^T = k^T q layout [m, n]; AV contraction over m
    needs no transposes.
  - Softmax rowsums via a ones-block packed into the AV stationary operand
    ([v_lo|ones] -> head-lo out on partitions 0:64, rowsum replicated on
    64:128; [ones|v_hi] mirrors for head-hi).
  - Normalization: reciprocal_approx_fast, DMA partition-shift, one multiply.

v2 scheduling: phases are interleaved across the two batches so the
Activation engine (exp, the per-core bottleneck at ~66us) never starves:
  gn(b0) qkv(b0) gn(b1) | attn(b0,hp0) | qkv(b1) | attn(b0,hp1)
  | attn(b1,hp0) | proj(b0) | attn(b1,hp1) | proj(b1)
GN/QKV/proj PE+DVE work rides in the act-engine shadow of attention.
"""
import numpy as np

B, C, HW = 16, 256, 1024
NH, D, NG = 4, 64, 8
EPS = 1e-5
NCORES = 8
BPC = B // NCORES  # batches per core

_CACHE = {}


def _build_module(reps=1):
    import concourse.bacc as bacc
    import concourse.mybir as mybir
    from concourse import tile

    f32 = mybir.dt.float32
    f32r = mybir.dt.float32r
    AF = mybir.ActivationFunctionType

    nc = bacc.Bacc("TRN2", target_bir_lowering=False, num_devices=NCORES)

    x_d = nc.dram_tensor("x", [BPC, C, HW], f32, kind="ExternalInput")
    qkvwT_d = nc.dram_tensor("qkvwT", [C, 3 * C], f32, kind="ExternalInput")
    projwT_d = nc.dram_tensor("projwT", [C, C], f32, kind="ExternalInput")
    qkb_d = nc.dram_tensor("qkb", [2 * C], f32, kind="ExternalInput")
    pb2_d = nc.dram_tensor("pb2", [C], f32, kind="ExternalInput")
    nw_d = nc.dram_tensor("nw", [C], f32, kind="ExternalInput")
    nb_d = nc.dram_tensor("nb", [C], f32, kind="ExternalInput")
    y_d = nc.dram_tensor("y", [BPC, C, HW], f32, kind="ExternalOutput")

    # constants: group indicator matrices + ones block
    # chunk ch covers channels [128*ch, 128*ch+128) -> groups [4*ch, 4*ch+4)
    g_np = np.zeros((2, 128, NG), np.float32)
    gb_np = np.zeros((2, NG, 128), np.float32)
    for ch in range(2):
        for c in range(128):
            g = 4 * ch + c // 32
            g_np[ch, c, g] = 1.0
            gb_np[ch, g, c] = 1.0
    g_dram = nc.inline_tensor(np.ascontiguousarray(g_np), name="g_const")
    gb_dram = nc.inline_tensor(np.ascontiguousarray(gb_np), name="gb_const")
    ones_dram = nc.inline_tensor(np.ones((128, 64), np.float32), name="ones_const")

    with tile.TileContext(nc) as tc:
        with (
            tc.tile_pool(name="wp", bufs=1) as wp,        # weights/consts, persistent
            tc.tile_pool(name="big", bufs=1) as big,      # per-batch persistent tiles
            tc.tile_pool(name="tmp", bufs=3) as tmp,      # small transient tiles
            tc.tile_pool(name="es_p", bufs=6) as es_p,    # exp(S^T) tiles
            tc.tile_pool(name="rec_p", bufs=3) as rec_p,  # recip tiles
            tc.tile_pool(name="y_p", bufs=2) as y_p,      # output staging
            tc.tile_pool(name="x_p", bufs=2) as x_p,      # input, double-buffered across reps
            # PSUM (16KB/partition): S stream 2x4KB, AV accumulator 1x4KB,
            # aux (qk/y/vt/GN tiles, <=2KB each) 2x2KB.
            tc.tile_pool(name="ps_s", bufs=2, space="PSUM") as ps_s,
            tc.tile_pool(name="ps_av", bufs=1, space="PSUM") as ps_av,
            tc.tile_pool(name="ps_aux", bufs=2, space="PSUM") as ps_aux,
        ):
            # ---------------- weights / constants ----------------
            qkvwT32 = wp.tile([128, 2, 3 * C], f32)
            projwT32 = wp.tile([128, 2, C], f32)
            for ch in range(2):
                nc.gpsimd.dma_start(qkvwT32[:, ch, :], qkvwT_d[128 * ch:128 * (ch + 1), :])
                nc.gpsimd.dma_start(projwT32[:, ch, :], projwT_d[128 * ch:128 * (ch + 1), :])
            qkvwT = wp.tile([128, 2, 3 * C], f32r)
            projwT = wp.tile([128, 2, C], f32r)
            nc.vector.tensor_copy(qkvwT[:], qkvwT32[:])
            nc.vector.tensor_copy(projwT[:], projwT32[:])

            qkb = wp.tile([128, 4], f32)
            nc.gpsimd.dma_start(qkb[:], qkb_d.rearrange("(t p) -> p t", p=128))
            pb2 = wp.tile([128, 2], f32)
            nc.gpsimd.dma_start(pb2[:], pb2_d.rearrange("(t p) -> p t", p=128))
            nw = wp.tile([128, 2], f32)
            nc.gpsimd.dma_start(nw[:], nw_d.rearrange("(t p) -> p t", p=128))
            nb = wp.tile([128, 2], f32)
            nc.gpsimd.dma_start(nb[:], nb_d.rearrange("(t p) -> p t", p=128))

            g_c = wp.tile([128, 2, NG], f32)
            nc.gpsimd.dma_start(g_c[:], g_dram[:].rearrange("c p g -> p c g"))
            gb_c = wp.tile([NG, 2, 128], f32)
            nc.gpsimd.dma_start(gb_c[:], gb_dram[:].rearrange("c p g -> p c g"))
            ones32 = wp.tile([128, 64], f32)
            nc.gpsimd.dma_start(ones32[:], ones_dram[:])
            ones_r = wp.tile([128, 64], f32r)
            nc.vector.tensor_copy(ones_r[:], ones32[:])
            eps_t = wp.tile([128, 1], f32)
            nc.vector.memset(eps_t[:], EPS)

            # persistent per-batch tiles
            h_t = big.tile([128, BPC, 2, HW], f32r)      # groupnorm output
            qk_t = big.tile([128, BPC, 4, HW], f32r)     # q01,q23,k01,k23
            vtp_t = big.tile([128, BPC, 2, 8, 192], f32r)  # [v_lo|ones|v_hi] per (hp, m-tile)
            on_t = big.tile([128, BPC, 2, HW], f32r)     # normalized attn out (pre-proj)

            def emit_gn(rep, b, x_t):
                """DMA x[b], GroupNorm stats + apply -> h_t[:, b]."""
                for ch in range(2):
                    nc.sync.dma_start(x_t[:, b, ch, :], x_d[b, 128 * ch:128 * (ch + 1), :])

                # per-channel stats -> per-group via PE -> broadcast back
                g_ps = ps_aux.tile([NG, 2], f32, name=f"g_ps_{rep}_{b}", tag="aux")
                st2s = []
                for ch in range(2):
                    st6 = tmp.tile([128, 2, 6], f32, name=f"st6_{rep}_{b}_{ch}", tag="st6")
                    for i in range(2):
                        nc.vector.bn_stats(st6[:, i, :], x_t[:, b, ch, 512 * i:512 * (i + 1)])
                    mv = tmp.tile([128, 2], f32, name=f"mv_{rep}_{b}_{ch}", tag="mv")
                    nc.vector.bn_aggr(mv[:], st6[:])
                    st2 = tmp.tile([128, 2], f32, name=f"st2_{rep}_{b}_{ch}", tag="st2")
                    nc.gpsimd.tensor_copy(st2[:, 0:1], mv[:, 0:1])
                    sq = tmp.tile([128, 1], f32, name=f"sq_{rep}_{b}_{ch}", tag="sq")
                    nc.vector.tensor_mul(sq[:], mv[:, 0:1], mv[:, 0:1])
                    nc.vector.tensor_add(st2[:, 1:2], mv[:, 1:2], sq[:])
                    st2s.append(st2)
                for ch in range(2):
                    nc.tensor.matmul(g_ps[:], g_c[:, ch, :], st2s[ch][:],
                                     start=(ch == 0), stop=(ch == 1))
                gst = tmp.tile([NG, 2], f32, name=f"gst_{rep}_{b}", tag="gst")
                nc.vector.tensor_copy(gst[:], g_ps[:])

                for ch in range(2):
                    bc_ps = ps_aux.tile([128, 2], f32, name=f"bc_ps_{rep}_{b}_{ch}", tag="aux")
                    nc.tensor.matmul(bc_ps[:], gb_c[:, ch, :], gst[:],
                                     start=True, stop=True)
                    mean_c = tmp.tile([128, 1], f32, name=f"mean_{rep}_{b}_{ch}", tag="mean")
                    ex2_c = tmp.tile([128, 1], f32, name=f"ex2_{rep}_{b}_{ch}", tag="ex2")
                    nc.vector.tensor_scalar_mul(mean_c[:], bc_ps[:, 0:1], 1.0 / 32.0)
                    nc.vector.tensor_scalar_mul(ex2_c[:], bc_ps[:, 1:2], 1.0 / 32.0)
                    var_c = tmp.tile([128, 1], f32, name=f"var_{rep}_{b}_{ch}", tag="var")
                    nc.vector.tensor_mul(var_c[:], mean_c[:], mean_c[:])
                    nc.vector.tensor_sub(var_c[:], ex2_c[:], var_c[:])
                    sd = tmp.tile([128, 1], f32, name=f"sd_{rep}_{b}_{ch}", tag="sd")
                    nc.scalar.activation(sd[:], var_c[:], AF.Sqrt, bias=eps_t[:])
                    nc.vector.reciprocal(sd[:], sd[:])
                    a_c = tmp.tile([128, 1], f32, name=f"a_{rep}_{b}_{ch}", tag="a_c")
                    nc.vector.tensor_mul(a_c[:], sd[:], nw[:, ch:ch + 1])
                    b_c = tmp.tile([128, 1], f32, name=f"b_{rep}_{b}_{ch}", tag="b_c")
                    nc.vector.tensor_mul(b_c[:], mean_c[:], a_c[:])
                    nc.vector.tensor_sub(b_c[:], nb[:, ch:ch + 1], b_c[:])
                    # GN apply on the act engine (Identity is in the exp table,
                    # so no act-table reload): h = x*a_c + b_c
                    nc.scalar.activation(
                        h_t[:, b, ch, :], x_t[:, b, ch, :], AF.Identity,
                        scale=a_c[:], bias=b_c[:])

            def emit_qk_tile(rep, b, t):
                """One 128-row output tile of q/k (t in 0..3) -> qk_t[:, b, t]."""
                for half in range(2):
                    qk_ps = ps_aux.tile([128, 512], f32,
                                        name=f"qk_ps_{rep}_{b}_{t}_{half}", tag="aux")
                    for ch in range(2):
                        nc.tensor.matmul(
                            qk_ps[:],
                            qkvwT[:, ch, 128 * t:128 * (t + 1)],
                            h_t[:, b, ch, 512 * half:512 * (half + 1)],
                            start=(ch == 0), stop=(ch == 1))
                    nc.scalar.activation(
                        qk_t[:, b, t, 512 * half:512 * (half + 1)], qk_ps[:],
                        AF.Identity, bias=qkb[:, t:t + 1])

            def emit_v(rep, b):
                """v^T tiles -> vtp_t[:, b] ([v_lo|ones|v_hi] per (hp, m-tile))."""
                for nt in range(8):
                    vt_ps = ps_aux.tile([128, 256], f32, name=f"vt_ps_{rep}_{b}_{nt}", tag="aux")
                    for ch in range(2):
                        nc.tensor.matmul(
                            vt_ps[:],
                            h_t[:, b, ch, 128 * nt:128 * (nt + 1)],
                            qkvwT[:, ch, 2 * C:3 * C],
                            start=(ch == 0), stop=(ch == 1))
                    for hp in range(2):
                        nc.scalar.copy(
                            vtp_t[:, b, hp, nt, :].rearrange("p (s c) -> p s c", s=3)[:, 0::2, :],
                            vt_ps[:, 128 * hp:128 * (hp + 1)].rearrange("p (s c) -> p s c", s=2))
                        nc.gpsimd.tensor_copy(vtp_t[:, b, hp, nt, 64:128], ones_r[:])

            def emit_attn(rep, b, hp):
                """Attention for (b, head-pair hp): S^T -> exp -> AV -> normalize."""
                q_ap = qk_t[:, b, hp, :]
                k_ap = qk_t[:, b, 2 + hp, :]
                for half in range(2):
                    av_ps = ps_av.tile([128, HW], f32, name=f"av_{rep}_{b}_{hp}_{half}",
                                       tag="av")
                    for m in range(8):
                        s_ps = ps_s.tile([128, HW], f32,
                                         name=f"s_{rep}_{b}_{hp}_{half}_{m}", tag="s")
                        nc.tensor.matmul(
                            s_ps[:, 0:512],
                            k_ap[0:64, 128 * m:128 * (m + 1)],
                            q_ap[0:64, 512 * half:512 * (half + 1)],
                            start=True, stop=True)
                        nc.tensor.matmul(
                            s_ps[:, 512:1024],
                            k_ap[64:128, 128 * m:128 * (m + 1)],
                            q_ap[64:128, 512 * half:512 * (half + 1)],
                            start=True, stop=True)
                        es = es_p.tile([128, HW], f32r,
                                       name=f"es_{rep}_{b}_{hp}_{half}_{m}", tag="es")
                        nc.scalar.activation(es[:], s_ps[:], AF.Exp, scale=0.125)
                        nc.tensor.matmul(
                            av_ps[:, 0:512], vtp_t[:, b, hp, m, 0:128], es[:, 0:512],
                            start=(m == 0), stop=(m == 7))
                        nc.tensor.matmul(
                            av_ps[:, 512:1024], vtp_t[:, b, hp, m, 64:192], es[:, 512:1024],
                            start=(m == 0), stop=(m == 7))
                    # normalize: rowsum_lo replicated on rows 64:128 of cols
                    # 0:512, rowsum_hi on rows 0:64 of cols 512:1024
                    rec = rec_p.tile([128, HW], f32, name=f"rl_{rep}_{b}_{hp}_{half}",
                                     tag="rl")
                    nc.vector.reciprocal_approx_fast(rec[:], av_ps[:])
                    recs = rec_p.tile([128, 512], f32, name=f"rs_{rep}_{b}_{hp}_{half}",
                                      tag="rs")
                    nc.sync.dma_start(recs[0:64, :], rec[64:128, 0:512])
                    nc.sync.dma_start(recs[64:128, :], rec[0:64, 512:1024])
                    nc.vector.tensor_mul(
                        on_t[0:64, b, hp, 512 * half:512 * (half + 1)],
                        av_ps[0:64, 0:512], recs[0:64, :])
                    nc.vector.tensor_mul(
                        on_t[64:128, b, hp, 512 * half:512 * (half + 1)],
                        av_ps[64:128, 512:1024], recs[64:128, :])

            def emit_proj(rep, b, x_t):
                """proj + bias + residual -> y DMA for batch b."""
                for ot in range(2):
                    y_sb = y_p.tile([128, HW], f32, name=f"y_sb_{rep}_{b}_{ot}", tag="y")
                    for half in range(2):
                        y_ps = ps_aux.tile([128, 512], f32,
                                           name=f"y_ps_{rep}_{b}_{ot}_{half}", tag="aux")
                        for ch in range(2):
                            nc.tensor.matmul(
                                y_ps[:],
                                projwT[:, ch, 128 * ot:128 * (ot + 1)],
                                on_t[:, b, ch, 512 * half:512 * (half + 1)],
                                start=(ch == 0), stop=(ch == 1))
                        nc.vector.scalar_tensor_tensor(
                            out=y_sb[:, 512 * half:512 * (half + 1)], in0=y_ps[:],
                            scalar=pb2[:, ot:ot + 1],
                            in1=x_t[:, b, ot, 512 * half:512 * (half + 1)],
                            op0=mybir.AluOpType.add, op1=mybir.AluOpType.add)
                    eng = nc.sync if ot == 0 else nc.gpsimd
                    eng.dma_start(y_d[b, 128 * ot:128 * (ot + 1), :], y_sb[:])

            for rep in range(reps):
                x_t = x_p.tile([128, BPC, 2, HW], f32, name=f"x_{rep}", tag="x")
                # Interleaved schedule: attention (act-engine-bound) overlaps
                # the other batch's GN/QKV and the previous batch's proj.
                emit_gn(rep, 0, x_t)
                emit_qk_tile(rep, 0, 0)   # q01
                emit_qk_tile(rep, 0, 2)   # k01
                emit_v(rep, 0)
                emit_qk_tile(rep, 0, 1)   # q23
                emit_qk_tile(rep, 0, 3)   # k23
                emit_gn(rep, 1, x_t)
                emit_attn(rep, 0, 0)
                emit_attn(rep, 0, 1)
                emit_qk_tile(rep, 1, 0)
                emit_qk_tile(rep, 1, 2)
                emit_v(rep, 1)
                emit_qk_tile(rep, 1, 1)
                emit_qk_tile(rep, 1, 3)
                emit_attn(rep, 1, 0)
                emit_proj(rep, 0, x_t)
                emit_attn(rep, 1, 1)
                emit_proj(rep, 1, x_t)

    nc.finalize()
    return nc


def _prep_inputs(x, norm_w, norm_b, qkv_w, qkv_b, proj_w, proj_b):
    x = np.asarray(x, np.float32).reshape(B, C, HW)
    qkv_w = np.asarray(qkv_w, np.float32)
    qkv_b = np.asarray(qkv_b, np.float32)
    proj_w = np.asarray(proj_w, np.float32)
    proj_b = np.asarray(proj_b, np.float32)
    qkvwT = np.ascontiguousarray(qkv_w.T)
    projwT = np.ascontiguousarray(proj_w.T)
    qkb = np.ascontiguousarray(qkv_b[:2 * C])
    # v-bias and proj bias folded: y += proj_w @ (out + v_bias) + proj_b
    pb2 = (np.asarray(proj_b, np.float64)
           + np.asarray(proj_w, np.float64) @ np.asarray(qkv_b[2 * C:], np.float64)
           ).astype(np.float32)
    shared = {
        "qkvwT": qkvwT, "projwT": projwT, "qkb": qkb, "pb2": pb2,
        "nw": np.ascontiguousarray(np.asarray(norm_w, np.float32)),
        "nb": np.ascontiguousarray(np.asarray(norm_b, np.float32)),
    }
    in_maps = []
    for i in range(NCORES):
        m = {"x": np.ascontiguousarray(x[BPC * i:BPC * (i + 1)])}
        m.update(shared)
        in_maps.append(m)
    return in_maps


def kernel(x, norm_w, norm_b, qkv_w, qkv_b, proj_w, proj_b, _profile=False, _reps=1):
    from concourse.bass_utils import run_bass_kernel_spmd

    key = ("nc", _reps)
    if key not in _CACHE:
        _CACHE[key] = _build_module(reps=_reps)
    nc = _CACHE[key]

    in_maps = _prep_inputs(x, norm_w, norm_b, qkv_w, qkv_b, proj_w, proj_b)
    res = run_bass_kernel_spmd(nc, in_maps, core_ids=list(range(NCORES)),
                               trace=_profile)
    y = np.concatenate([r["y"] for r in res.results], axis=0)
    y = y.reshape(B, C, 32, 32)
    if _profile:
        return y, res
    return y
